# revision 7
# baseline (speedup 1.0000x reference)
"""Trainium2 Bass kernel for nn_CAML_53240414601378.

Embedding lookup -> Conv1d(k=4, pad=2) -> tanh -> per-label attention
pooling -> logits. Data-parallel over batch across 8 NeuronCores
(4 batches per core); small params replicated.

Math note: logits[b,l] = sum_s alpha[b,s,l] * t[b,l,s] + final_b[l]
where t = final_w @ H^T -- the (B,L,F) intermediate of the reference is
never materialized.

Self-contained: hardcodes all shapes from the problem spec.
"""

import numpy as np
import ml_dtypes

import concourse.bass as bass
import concourse.tile as tile
from concourse import bacc, mybir
from concourse.bass_utils import run_bass_kernel_spmd

B, S = 32, 4096
VOCAB, E, F, L = 30522, 256, 256, 50
SO = S + 1  # conv output length (4097)
N_CORES = 8
BPC = B // N_CORES  # batches per core
BF16 = mybir.dt.bfloat16
FP32 = mybir.dt.float32

_cache = {}


def build_nc():
    nc = bacc.Bacc("TRN2", target_bir_lowering=False, debug=False,
                   num_devices=N_CORES)

    emb_ap = nc.dram_tensor("emb", (VOCAB, E), BF16, kind="ExternalInput").ap()
    idx_ap = nc.dram_tensor("idx", (128, BPC * S // 16), mybir.dt.int16,
                            kind="ExternalInput").ap()
    w_ap = nc.dram_tensor("wconv", (128, 16, 128), BF16,
                          kind="ExternalInput").ap()
    uw_ap = nc.dram_tensor("uwfw", (128, 2, 2 * L), BF16,
                           kind="ExternalInput").ap()
    cb_ap = nc.dram_tensor("cbias", (128, 2), FP32, kind="ExternalInput").ap()
    fb_ap = nc.dram_tensor("fbias", (L, 1), FP32, kind="ExternalInput").ap()
    out_ap = nc.dram_tensor("out", (L, BPC), FP32, kind="ExternalOutput").ap()

    with tile.TileContext(nc) as tc:
        with (
            tc.tile_pool(name="const", bufs=1) as const,
            tc.tile_pool(name="xp", bufs=2) as xp,
            tc.tile_pool(name="hp", bufs=2) as hp,
            tc.tile_pool(name="stp", bufs=2) as stp,
            tc.tile_pool(name="small", bufs=8) as small,
            tc.tile_pool(name="psum", bufs=2, space="PSUM") as psum,
        ):
            # ---- constants (loaded once) ----
            w_sb = const.tile([128, 16, 128], BF16)
            nc.sync.dma_start(w_sb[:], w_ap[:])
            uw_sb = const.tile([128, 2, 2 * L], BF16)
            nc.sync.dma_start(uw_sb[:], uw_ap[:])
            cb_sb = const.tile([128, 2], FP32)
            nc.sync.dma_start(cb_sb[:], cb_ap[:])
            fb_sb = const.tile([L, 1], FP32)
            nc.sync.dma_start(fb_sb[:], fb_ap[:])
            idx_sb = const.tile([128, BPC * S // 16], mybir.dt.int16)
            nc.sync.dma_start(idx_sb[:], idx_ap[:])
            out_sb = const.tile([L, BPC], FP32)

            NT = 8          # full seq tiles of 512 covering t in [0, 4096)
            TN = 512

            for b in range(BPC):
                # ---- embedding gather, transposed to (e%128, e//128, s) ----
                x_t = xp.tile([128, 2, S], BF16, tag="x")
                nc.gpsimd.dma_gather(
                    out_ap=x_t[:],
                    in_ap=emb_ap[:],
                    idxs_ap=idx_sb[:, b * (S // 16):(b + 1) * (S // 16)],
                    num_idxs=S,
                    num_idxs_reg=S,
                    elem_size=E,
                    transpose=True,
                    single_packet=False,
                )

                # ---- conv1d(k=4, pad=2) + tanh -> H (f%128, f//128, t) ----
                H = hp.tile([128, 2, SO], BF16, tag="H")
                for fc in range(2):
                    for j in range(NT):
                        t0 = j * TN
                        ph = psum.tile([128, TN], FP32, tag=f"h{fc}")
                        # shifts, full-width ones first so start=True
                        # covers the whole bank
                        shifts = []
                        for k in range(4):
                            lo = max(0, t0 + k - 2)
                            hi = min(S, t0 + k - 2 + TN)
                            shifts.append((k, lo, hi, lo - (t0 + k - 2)))
                        shifts.sort(key=lambda s: -(s[2] - s[1]))
                        n_mm = len(shifts) * 2
                        i = 0
                        for (k, lo, hi, off) in shifts:
                            for ec in range(2):
                                nc.tensor.matmul(
                                    ph[:, off:off + (hi - lo)],
                                    w_sb[:, k * 4 + ec * 2 + fc, :],
                                    x_t[:, ec, lo:hi],
                                    start=(i == 0),
                                    stop=(i == n_mm - 1),
                                )
                                i += 1
                        nc.scalar.activation(
                            H[:, fc, t0:t0 + TN], ph[:],
                            mybir.ActivationFunctionType.Tanh,
                            bias=cb_sb[:, fc:fc + 1],
                        )
                    # last output column t = 4096: x cols 4094..4097,
                    # only k=0,1 in range
                    ph9 = psum.tile([128, 1], FP32, tag=f"h{fc}")
                    i = 0
                    for k in range(2):
                        for ec in range(2):
                            nc.tensor.matmul(
                                ph9[:, 0:1],
                                w_sb[:, k * 4 + ec * 2 + fc, :],
                                x_t[:, ec, S - 2 + k:S - 1 + k],
                                start=(i == 0),
                                stop=(i == 3),
                            )
                            i += 1
                    nc.scalar.activation(
                        H[:, fc, S:SO], ph9[:],
                        mybir.ActivationFunctionType.Tanh,
                        bias=cb_sb[:, fc:fc + 1],
                    )

                # ---- scores = U_w @ H, t = final_w @ H  (L, seq) ----
                sc_sb = stp.tile([L, SO], FP32, tag="sc")
                tt_sb = stp.tile([L, SO], FP32, tag="tt")
                for j in range(NT + 1):
                    t0 = j * TN
                    n = TN if j < NT else 1
                    ps = psum.tile([L, n], FP32, tag="s")
                    pt = psum.tile([L, n], FP32, tag="t")
                    for fc in range(2):
                        nc.tensor.matmul(
                            ps[:], uw_sb[:, fc, 0:L], H[:, fc, t0:t0 + n],
                            start=(fc == 0), stop=(fc == 1),
                        )
                    for fc in range(2):
                        nc.tensor.matmul(
                            pt[:], uw_sb[:, fc, L:2 * L], H[:, fc, t0:t0 + n],
                            start=(fc == 0), stop=(fc == 1),
                        )
                    nc.vector.tensor_copy(sc_sb[:, t0:t0 + n], ps[:])
                    nc.scalar.copy(tt_sb[:, t0:t0 + n], pt[:])

                # ---- softmax over seq + weighted sum -> logits ----
                mx = small.tile([L, 1], FP32, tag="mx")
                nc.vector.reduce_max(mx[:], sc_sb[:], axis=mybir.AxisListType.X)
                nmx = small.tile([L, 1], FP32, tag="nmx")
                nc.scalar.mul(nmx[:], mx[:], -1.0)
                zsum = small.tile([L, 1], FP32, tag="z")
                nc.scalar.activation(
                    sc_sb[:], sc_sb[:], mybir.ActivationFunctionType.Exp,
                    bias=nmx[:], accum_out=zsum[:],
                )
                num = small.tile([L, 1], FP32, tag="num")
                nc.vector.tensor_mul(tt_sb[:], sc_sb[:], tt_sb[:])
                nc.vector.reduce_sum(num[:], tt_sb[:],
                                     axis=mybir.AxisListType.X)
                zr = small.tile([L, 1], FP32, tag="zr")
                nc.vector.reciprocal(zr[:], zsum[:])
                sm = small.tile([L, 1], FP32, tag="sm")
                nc.vector.tensor_mul(sm[:], num[:], zr[:])
                nc.vector.tensor_add(out_sb[:, b:b + 1], sm[:], fb_sb[:])

            nc.sync.dma_start(out_ap[:], out_sb[:])

    nc.compile()
    return nc


def _prep_shared(emb_table, conv_w, conv_b, U_w, final_w, final_b):
    emb_bf = np.ascontiguousarray(emb_table.astype(ml_dtypes.bfloat16))

    # wconv[e_lo, k*4 + ec*2 + fc, f_lo] = conv_w[fc*128+f, ec*128+e, k]
    W = np.empty((128, 16, 128), np.float32)
    for k in range(4):
        for ec in range(2):
            for fc in range(2):
                W[:, k * 4 + ec * 2 + fc, :] = conv_w[
                    fc * 128:(fc + 1) * 128, ec * 128:(ec + 1) * 128, k].T
    W = np.ascontiguousarray(W.astype(ml_dtypes.bfloat16))

    # uwfw[f_lo, fc, j]: j<L -> U_w[j, fc*128+f_lo], else final_w
    uT = U_w.T.reshape(2, 128, L).transpose(1, 0, 2)
    fT = final_w.T.reshape(2, 128, L).transpose(1, 0, 2)
    UW = np.ascontiguousarray(
        np.concatenate([uT, fT], axis=2).astype(ml_dtypes.bfloat16))

    CB = np.ascontiguousarray(conv_b.reshape(2, 128).T.astype(np.float32))
    FB = np.ascontiguousarray(final_b.reshape(L, 1).astype(np.float32))
    return emb_bf, W, UW, CB, FB


def kernel(input_ids, emb_table, conv_w, conv_b, U_w, final_w, final_b):
    import os
    ids = np.asarray(input_ids)
    emb_table = np.asarray(emb_table, dtype=np.float32)
    conv_w = np.asarray(conv_w, dtype=np.float32)
    conv_b = np.asarray(conv_b, dtype=np.float32)
    U_w = np.asarray(U_w, dtype=np.float32)
    final_w = np.asarray(final_w, dtype=np.float32)
    final_b = np.asarray(final_b, dtype=np.float32)

    if "nc" not in _cache:
        _cache["nc"] = build_nc()
    nc = _cache["nc"]

    emb_bf, W, UW, CB, FB = _prep_shared(
        emb_table, conv_w, conv_b, U_w, final_w, final_b)

    ids16 = ids.astype(np.int16)  # vocab 30522 < 2**15
    in_maps = []
    for c in range(N_CORES):
        cid = ids16[c * BPC:(c + 1) * BPC]  # (BPC, S)
        # position i -> [i % 16, i // 16], batches along axis 1; the
        # 16-row block is replicated to all 8 gpsimd cores (128 rows)
        blk = np.concatenate(
            [cid[b].reshape(S // 16, 16).T for b in range(BPC)], axis=1)
        idx = np.tile(blk, (8, 1))
        in_maps.append({
            "emb": emb_bf, "idx": np.ascontiguousarray(idx),
            "wconv": W, "uwfw": UW, "cbias": CB, "fbias": FB,
        })

    trace = bool(int(os.environ.get("KERNEL_TRACE", "0")))
    res = run_bass_kernel_spmd(nc, in_maps, core_ids=list(range(N_CORES)),
                               trace=trace)
    _cache["last_result"] = res

    out = np.concatenate(
        [res.results[c]["out"].T for c in range(N_CORES)], axis=0)
    return np.ascontiguousarray(out.astype(np.float32))


# revision 9
# speedup vs baseline: 1.0171x; 1.0171x over previous
"""Trainium2 Bass kernel for nn_CAML_53240414601378.

Embedding lookup -> Conv1d(k=4, pad=2) -> tanh -> per-label attention
pooling -> logits. Data-parallel over batch across 8 NeuronCores
(4 batches per core); small params replicated.

Math note: logits[b,l] = sum_s alpha[b,s,l] * t[b,l,s] + final_b[l]
where t = final_w @ H^T -- the (B,L,F) intermediate of the reference is
never materialized. scores and t come from ONE matmul with a combined
stationary operand (U_w in psum rows 0..49, final_w in rows 64..113).

Self-contained: hardcodes all shapes from the problem spec.
"""

import numpy as np
import ml_dtypes

import concourse.bass as bass
import concourse.tile as tile
from concourse import bacc, mybir
from concourse.bass_utils import run_bass_kernel_spmd

B, S = 32, 4096
VOCAB, E, F, L = 30522, 256, 256, 50
SO = S + 1  # conv output length (4097)
N_CORES = 8
BPC = B // N_CORES  # batches per core
BF16 = mybir.dt.bfloat16
FP32 = mybir.dt.float32
NT, TN = 8, 512  # full seq tiles covering t in [0, 4096)

_cache = {}


def _conv_mms(t0, n):
    """Matmul pieces for conv output cols [t0, t0+n): (k, lo, hi, off),
    full-width first so start=True covers the whole psum range."""
    shifts = []
    for k in range(4):
        lo = max(0, t0 + k - 2)
        hi = min(S, t0 + k - 2 + n)
        shifts.append((k, lo, hi, lo - (t0 + k - 2)))
    shifts.sort(key=lambda s: -(s[2] - s[1]))
    return shifts


def build_nc():
    nc = bacc.Bacc("TRN2", target_bir_lowering=False, debug=False,
                   num_devices=N_CORES)

    emb_ap = nc.dram_tensor("emb", (VOCAB, E), BF16, kind="ExternalInput").ap()
    idx_ap = nc.dram_tensor("idx", (128, BPC * S // 16), mybir.dt.int16,
                            kind="ExternalInput").ap()
    w_ap = nc.dram_tensor("wconv", (128, 16, 128), BF16,
                          kind="ExternalInput").ap()
    uw_ap = nc.dram_tensor("uwfw", (128, 2, 114), BF16,
                           kind="ExternalInput").ap()
    cb_ap = nc.dram_tensor("cbias", (128, 2), FP32, kind="ExternalInput").ap()
    fb_ap = nc.dram_tensor("fbias", (L, 1), FP32, kind="ExternalInput").ap()
    out_ap = nc.dram_tensor("out", (L, BPC), FP32, kind="ExternalOutput").ap()

    with tile.TileContext(nc) as tc:
        with (
            tc.tile_pool(name="const", bufs=1) as const,
            tc.tile_pool(name="xp", bufs=3) as xp,
            tc.tile_pool(name="hp", bufs=2) as hp,
            tc.tile_pool(name="stp", bufs=2) as stp,
            tc.tile_pool(name="small", bufs=8) as small,
            tc.tile_pool(name="psum", bufs=2, space="PSUM") as psum,
            tc.tile_pool(name="psum_st", bufs=4, space="PSUM") as psum_st,
        ):
            # ---- constants (loaded once) ----
            idx_sb = const.tile([128, BPC * S // 16], mybir.dt.int16)
            nc.sync.dma_start(idx_sb[:], idx_ap[:])
            w_sb = const.tile([128, 16, 128], BF16)
            nc.sync.dma_start(w_sb[:], w_ap[:])
            uw_sb = const.tile([128, 2, 114], BF16)
            nc.sync.dma_start(uw_sb[:], uw_ap[:])
            cb_sb = const.tile([128, 2], FP32)
            nc.sync.dma_start(cb_sb[:], cb_ap[:])
            fb_sb = const.tile([L, 1], FP32)
            nc.sync.dma_start(fb_sb[:], fb_ap[:])
            out_sb = const.tile([L, BPC], FP32)

            IPB = S // 16  # idx columns per batch

            for b in range(BPC):
                # ---- embedding gather -> (e%128, e//128, s), bf16 ----
                # batch 0 is gathered in two halves on the two SWDGE
                # queues so conv can start after the first half; other
                # batches use one gather, alternating queues.
                if b == 0:
                    HS = S // 2
                    xa = xp.tile([128, 2, HS], BF16, tag="xa")
                    xb = xp.tile([128, 2, HS], BF16, tag="xb")
                    for q, xt in ((0, xa), (1, xb)):
                        nc.gpsimd.dma_gather(
                            out_ap=xt[:], in_ap=emb_ap[:],
                            idxs_ap=idx_sb[:, q * (HS // 16):
                                           (q + 1) * (HS // 16)],
                            num_idxs=HS, num_idxs_reg=HS, elem_size=E,
                            transpose=True, single_packet=False)
                    segs = [(xa, 0, HS), (xb, HS, S)]
                else:
                    x_t = xp.tile([128, 2, S], BF16, tag="x")
                    nc.gpsimd.dma_gather(
                        out_ap=x_t[:], in_ap=emb_ap[:],
                        idxs_ap=idx_sb[:, b * IPB:(b + 1) * IPB],
                        num_idxs=S, num_idxs_reg=S, elem_size=E,
                        transpose=True, single_packet=False)
                    segs = [(x_t, 0, S)]

                def rhs_pieces(lo, hi):
                    """Split global x col range [lo,hi) by segment."""
                    out = []
                    for (xt, g0, g1) in segs:
                        a, bnd = max(lo, g0), min(hi, g1)
                        if a < bnd:
                            out.append((xt, a - g0, bnd - g0, a - lo))
                    return out

                # ---- conv1d(k=4) + bias + tanh -> H (f%128, f//128, t) ----
                H = hp.tile([128, 2, SO], BF16, tag="H")
                for fc in range(2):
                    for j in range(NT):
                        t0 = j * TN
                        ph = psum.tile([128, TN], FP32, tag=f"h{fc}")
                        mms = []
                        for (k, lo, hi, off) in _conv_mms(t0, TN):
                            for (xt, a, bnd, rel) in rhs_pieces(lo, hi):
                                for ec in range(2):
                                    mms.append((k, ec, xt, a, bnd, off + rel))
                        for i, (k, ec, xt, a, bnd, off) in enumerate(mms):
                            nc.tensor.matmul(
                                ph[:, off:off + (bnd - a)],
                                w_sb[:, k * 4 + ec * 2 + fc, :],
                                xt[:, ec, a:bnd],
                                start=(i == 0), stop=(i == len(mms) - 1),
                            )
                        nc.scalar.activation(
                            H[:, fc, t0:t0 + TN], ph[:],
                            mybir.ActivationFunctionType.Tanh,
                            bias=cb_sb[:, fc:fc + 1],
                        )
                    # last output column t = 4096: x cols 4094..4095,
                    # shifts k=0,1 only
                    ph9 = psum.tile([128, 1], FP32, tag=f"h{fc}")
                    xt, g0, _ = segs[-1]
                    i = 0
                    for k in range(2):
                        for ec in range(2):
                            nc.tensor.matmul(
                                ph9[:, 0:1],
                                w_sb[:, k * 4 + ec * 2 + fc, :],
                                xt[:, ec, S - 2 + k - g0:S - 1 + k - g0],
                                start=(i == 0), stop=(i == 3),
                            )
                            i += 1
                    nc.scalar.activation(
                        H[:, fc, S:SO], ph9[:],
                        mybir.ActivationFunctionType.Tanh,
                        bias=cb_sb[:, fc:fc + 1],
                    )

                # ---- scores (psum rows 0..49) & t (rows 64..113) ----
                sc_sb = stp.tile([L, SO], FP32, tag="sc")
                tt_sb = stp.tile([L, SO], FP32, tag="tt")
                for j in range(NT + 1):
                    t0 = j * TN
                    n = TN if j < NT else 1
                    pst = psum_st.tile([114, n], FP32, tag="st")
                    for fc in range(2):
                        nc.tensor.matmul(
                            pst[:], uw_sb[:, fc, :], H[:, fc, t0:t0 + n],
                            start=(fc == 0), stop=(fc == 1),
                        )
                    nc.vector.tensor_copy(sc_sb[:, t0:t0 + n], pst[0:L, :])
                    nc.scalar.copy(tt_sb[:, t0:t0 + n], pst[64:64 + L, :])

                # ---- softmax over seq + weighted sum -> logits ----
                mx = small.tile([L, 1], FP32, tag="mx")
                nc.vector.reduce_max(mx[:], sc_sb[:], axis=mybir.AxisListType.X)
                nmx = small.tile([L, 1], FP32, tag="nmx")
                nc.scalar.mul(nmx[:], mx[:], -1.0)
                zsum = small.tile([L, 1], FP32, tag="z")
                nc.scalar.activation(
                    sc_sb[:], sc_sb[:], mybir.ActivationFunctionType.Exp,
                    bias=nmx[:], accum_out=zsum[:],
                )
                num = small.tile([L, 1], FP32, tag="num")
                nc.vector.tensor_mul(tt_sb[:], sc_sb[:], tt_sb[:])
                nc.vector.reduce_sum(num[:], tt_sb[:],
                                     axis=mybir.AxisListType.X)
                zr = small.tile([L, 1], FP32, tag="zr")
                nc.vector.reciprocal(zr[:], zsum[:])
                sm = small.tile([L, 1], FP32, tag="sm")
                nc.vector.tensor_mul(sm[:], num[:], zr[:])
                nc.vector.tensor_add(out_sb[:, b:b + 1], sm[:], fb_sb[:])

            nc.sync.dma_start(out_ap[:], out_sb[:])

    nc.compile()
    return nc


def _prep_shared(emb_table, conv_w, conv_b, U_w, final_w, final_b):
    emb_bf = np.ascontiguousarray(emb_table.astype(ml_dtypes.bfloat16))

    # wconv[e_lo, k*4 + ec*2 + fc, f_lo] = conv_w[fc*128+f, ec*128+e, k]
    W = np.empty((128, 16, 128), np.float32)
    for k in range(4):
        for ec in range(2):
            for fc in range(2):
                W[:, k * 4 + ec * 2 + fc, :] = conv_w[
                    fc * 128:(fc + 1) * 128, ec * 128:(ec + 1) * 128, k].T
    W = np.ascontiguousarray(W.astype(ml_dtypes.bfloat16))

    # uwfw[f_lo, fc, j]: j<50 -> U_w[j, fc*128+f_lo];
    # j in [64,114) -> final_w[j-64, fc*128+f_lo]; rest zero
    UW = np.zeros((128, 2, 114), np.float32)
    UW[:, :, 0:L] = U_w.T.reshape(2, 128, L).transpose(1, 0, 2)
    UW[:, :, 64:64 + L] = final_w.T.reshape(2, 128, L).transpose(1, 0, 2)
    UW = np.ascontiguousarray(UW.astype(ml_dtypes.bfloat16))

    CB = np.ascontiguousarray(conv_b.reshape(2, 128).T.astype(np.float32))
    FB = np.ascontiguousarray(final_b.reshape(L, 1).astype(np.float32))
    return emb_bf, W, UW, CB, FB


def kernel(input_ids, emb_table, conv_w, conv_b, U_w, final_w, final_b):
    import os
    ids = np.asarray(input_ids)
    emb_table = np.asarray(emb_table, dtype=np.float32)
    conv_w = np.asarray(conv_w, dtype=np.float32)
    conv_b = np.asarray(conv_b, dtype=np.float32)
    U_w = np.asarray(U_w, dtype=np.float32)
    final_w = np.asarray(final_w, dtype=np.float32)
    final_b = np.asarray(final_b, dtype=np.float32)

    if "nc" not in _cache:
        _cache["nc"] = build_nc()
    nc = _cache["nc"]

    emb_bf, W, UW, CB, FB = _prep_shared(
        emb_table, conv_w, conv_b, U_w, final_w, final_b)

    ids16 = ids.astype(np.int16)  # vocab 30522 < 2**15
    in_maps = []
    for c in range(N_CORES):
        cid = ids16[c * BPC:(c + 1) * BPC]  # (BPC, S)
        # position i -> [i % 16, i // 16], batches along axis 1; the
        # 16-row block is replicated to all 8 gpsimd cores (128 rows)
        blk = np.concatenate(
            [cid[b].reshape(S // 16, 16).T for b in range(BPC)], axis=1)
        idx = np.tile(blk, (8, 1))
        in_maps.append({
            "emb": emb_bf, "idx": np.ascontiguousarray(idx),
            "wconv": W, "uwfw": UW, "cbias": CB, "fbias": FB,
        })

    trace = bool(int(os.environ.get("KERNEL_TRACE", "0")))
    res = run_bass_kernel_spmd(nc, in_maps, core_ids=list(range(N_CORES)),
                               trace=trace)
    _cache["last_result"] = res

    out = np.concatenate(
        [res.results[c]["out"].T for c in range(N_CORES)], axis=0)
    return np.ascontiguousarray(out.astype(np.float32))


# revision 11
# speedup vs baseline: 1.0397x; 1.0223x over previous
"""Trainium2 Bass kernel for nn_CAML_53240414601378.

Embedding lookup -> Conv1d(k=4, pad=2) -> tanh -> per-label attention
pooling -> logits. Data-parallel over batch across 8 NeuronCores
(4 batches per core); small params replicated.

Math note: logits[b,l] = sum_s alpha[b,s,l] * t[b,l,s] + final_b[l]
where t = final_w @ H^T -- the (B,L,F) intermediate of the reference is
never materialized. scores and t come from ONE matmul with a combined
stationary operand (U_w in psum rows 0..49, final_w in rows 64..113).

Self-contained: hardcodes all shapes from the problem spec.
"""

import numpy as np
import ml_dtypes

import concourse.bass as bass
import concourse.tile as tile
from concourse import bacc, mybir
from concourse.bass_utils import run_bass_kernel_spmd

B, S = 32, 4096
VOCAB, E, F, L = 30522, 256, 256, 50
SO = S + 1  # conv output length (4097)
N_CORES = 8
BPC = B // N_CORES  # batches per core
BF16 = mybir.dt.bfloat16
FP32 = mybir.dt.float32
NT, TN = 8, 512  # full seq tiles covering t in [0, 4096)

_cache = {}


def _conv_mms(t0, n):
    """Matmul pieces for conv output cols [t0, t0+n): (k, lo, hi, off),
    full-width first so start=True covers the whole psum range."""
    shifts = []
    for k in range(4):
        lo = max(0, t0 + k - 2)
        hi = min(S, t0 + k - 2 + n)
        shifts.append((k, lo, hi, lo - (t0 + k - 2)))
    shifts.sort(key=lambda s: -(s[2] - s[1]))
    return shifts


def build_nc():
    nc = bacc.Bacc("TRN2", target_bir_lowering=False, debug=False,
                   num_devices=N_CORES, dynamic_dma_scratch_size=32768)

    emb_ap = nc.dram_tensor("emb", (VOCAB, E), BF16, kind="ExternalInput").ap()
    idx_ap = nc.dram_tensor("idx", (128, BPC * S // 16), mybir.dt.int16,
                            kind="ExternalInput").ap()
    w_ap = nc.dram_tensor("wconv", (128, 16, 128), BF16,
                          kind="ExternalInput").ap()
    uw_ap = nc.dram_tensor("uwfw", (128, 2, 114), BF16,
                           kind="ExternalInput").ap()
    cb_ap = nc.dram_tensor("cbias", (128, 2), FP32, kind="ExternalInput").ap()
    fb_ap = nc.dram_tensor("fbias", (L, 1), FP32, kind="ExternalInput").ap()
    out_ap = nc.dram_tensor("out", (L, BPC), FP32, kind="ExternalOutput").ap()

    with tile.TileContext(nc) as tc:
        with (
            tc.tile_pool(name="const", bufs=1) as const,
            tc.tile_pool(name="xp0", bufs=1) as xp0,
            tc.tile_pool(name="xp", bufs=2) as xp,
            tc.tile_pool(name="hp", bufs=2) as hp,
            tc.tile_pool(name="stp", bufs=2) as stp,
            tc.tile_pool(name="small", bufs=8) as small,
            tc.tile_pool(name="psum", bufs=2, space="PSUM") as psum,
            tc.tile_pool(name="psum_st", bufs=4, space="PSUM") as psum_st,
        ):
            # ---- constants (loaded once) ----
            idx_sb = const.tile([128, BPC * S // 16], mybir.dt.int16)
            nc.sync.dma_start(idx_sb[:], idx_ap[:])
            w_sb = const.tile([128, 16, 128], BF16)
            nc.sync.dma_start(w_sb[:], w_ap[:])
            uw_sb = const.tile([128, 2, 114], BF16)
            nc.sync.dma_start(uw_sb[:], uw_ap[:])
            cb_sb = const.tile([128, 2], FP32)
            nc.sync.dma_start(cb_sb[:], cb_ap[:])
            fb_sb = const.tile([L, 1], FP32)
            nc.sync.dma_start(fb_sb[:], fb_ap[:])
            out_sb = const.tile([L, BPC], FP32)

            IPB = S // 16  # idx columns per batch

            for b in range(BPC):
                # ---- embedding gather -> (e%128, e//128, s), bf16 ----
                # batch 0 is gathered in two halves on the two SWDGE
                # queues so conv can start after the first half; other
                # batches use one gather, alternating queues.
                if b == 0:
                    HS = S // 2
                    xa = xp0.tile([128, 2, HS], BF16, tag="xa")
                    xb = xp0.tile([128, 2, HS], BF16, tag="xb")
                    for q, xt in ((0, xa), (1, xb)):
                        nc.gpsimd.dma_gather(
                            out_ap=xt[:], in_ap=emb_ap[:],
                            idxs_ap=idx_sb[:, q * (HS // 16):
                                           (q + 1) * (HS // 16)],
                            num_idxs=HS, num_idxs_reg=HS, elem_size=E,
                            transpose=True, single_packet=False)
                    segs = [(xa, 0, HS), (xb, HS, S)]
                else:
                    x_t = xp.tile([128, 2, S], BF16, tag="x")
                    nc.gpsimd.dma_gather(
                        out_ap=x_t[:], in_ap=emb_ap[:],
                        idxs_ap=idx_sb[:, b * IPB:(b + 1) * IPB],
                        num_idxs=S, num_idxs_reg=S, elem_size=E,
                        transpose=True, single_packet=False)
                    segs = [(x_t, 0, S)]

                def rhs_pieces(lo, hi):
                    """Split global x col range [lo,hi) by segment."""
                    out = []
                    for (xt, g0, g1) in segs:
                        a, bnd = max(lo, g0), min(hi, g1)
                        if a < bnd:
                            out.append((xt, a - g0, bnd - g0, a - lo))
                    return out

                # ---- conv1d(k=4) + bias + tanh -> H (f%128, f//128, t) ----
                H = hp.tile([128, 2, SO], BF16, tag="H")
                for fc in range(2):
                    for j in range(NT):
                        t0 = j * TN
                        ph = psum.tile([128, TN], FP32, tag=f"h{fc}")
                        mms = []
                        for (k, lo, hi, off) in _conv_mms(t0, TN):
                            for (xt, a, bnd, rel) in rhs_pieces(lo, hi):
                                for ec in range(2):
                                    mms.append((k, ec, xt, a, bnd, off + rel))
                        for i, (k, ec, xt, a, bnd, off) in enumerate(mms):
                            nc.tensor.matmul(
                                ph[:, off:off + (bnd - a)],
                                w_sb[:, k * 4 + ec * 2 + fc, :],
                                xt[:, ec, a:bnd],
                                start=(i == 0), stop=(i == len(mms) - 1),
                            )
                        nc.scalar.activation(
                            H[:, fc, t0:t0 + TN], ph[:],
                            mybir.ActivationFunctionType.Tanh,
                            bias=cb_sb[:, fc:fc + 1],
                        )
                    # last output column t = 4096: x cols 4094..4095,
                    # shifts k=0,1 only
                    ph9 = psum.tile([128, 1], FP32, tag=f"h{fc}")
                    xt, g0, _ = segs[-1]
                    i = 0
                    for k in range(2):
                        for ec in range(2):
                            nc.tensor.matmul(
                                ph9[:, 0:1],
                                w_sb[:, k * 4 + ec * 2 + fc, :],
                                xt[:, ec, S - 2 + k - g0:S - 1 + k - g0],
                                start=(i == 0), stop=(i == 3),
                            )
                            i += 1
                    nc.scalar.activation(
                        H[:, fc, S:SO], ph9[:],
                        mybir.ActivationFunctionType.Tanh,
                        bias=cb_sb[:, fc:fc + 1],
                    )

                # ---- scores (psum rows 0..49) & t (rows 64..113) ----
                sc_sb = stp.tile([L, SO], FP32, tag="sc")
                tt_sb = stp.tile([L, SO], BF16, tag="tt")
                for j in range(NT + 1):
                    t0 = j * TN
                    n = TN if j < NT else 1
                    pst = psum_st.tile([114, n], FP32, tag="st")
                    for fc in range(2):
                        nc.tensor.matmul(
                            pst[:], uw_sb[:, fc, :], H[:, fc, t0:t0 + n],
                            start=(fc == 0), stop=(fc == 1),
                        )
                    nc.vector.tensor_copy(sc_sb[:, t0:t0 + n], pst[0:L, :])
                    nc.scalar.copy(tt_sb[:, t0:t0 + n], pst[64:64 + L, :])

                # ---- softmax over seq + weighted sum -> logits ----
                mx = small.tile([L, 1], FP32, tag="mx")
                nc.vector.reduce_max(mx[:], sc_sb[:], axis=mybir.AxisListType.X)
                nmx = small.tile([L, 1], FP32, tag="nmx")
                nc.scalar.mul(nmx[:], mx[:], -1.0)
                zsum = small.tile([L, 1], FP32, tag="z")
                nc.scalar.activation(
                    sc_sb[:], sc_sb[:], mybir.ActivationFunctionType.Exp,
                    bias=nmx[:], accum_out=zsum[:],
                )
                num = small.tile([L, 1], FP32, tag="num")
                nc.vector.tensor_mul(sc_sb[:], sc_sb[:], tt_sb[:])
                nc.vector.reduce_sum(num[:], sc_sb[:],
                                     axis=mybir.AxisListType.X)
                zr = small.tile([L, 1], FP32, tag="zr")
                nc.vector.reciprocal(zr[:], zsum[:])
                sm = small.tile([L, 1], FP32, tag="sm")
                nc.vector.tensor_mul(sm[:], num[:], zr[:])
                nc.vector.tensor_add(out_sb[:, b:b + 1], sm[:], fb_sb[:])

            nc.sync.dma_start(out_ap[:], out_sb[:])

    nc.compile()
    return nc


def _prep_shared(emb_table, conv_w, conv_b, U_w, final_w, final_b):
    emb_bf = np.ascontiguousarray(emb_table.astype(ml_dtypes.bfloat16))

    # wconv[e_lo, k*4 + ec*2 + fc, f_lo] = conv_w[fc*128+f, ec*128+e, k]
    W = np.empty((128, 16, 128), np.float32)
    for k in range(4):
        for ec in range(2):
            for fc in range(2):
                W[:, k * 4 + ec * 2 + fc, :] = conv_w[
                    fc * 128:(fc + 1) * 128, ec * 128:(ec + 1) * 128, k].T
    W = np.ascontiguousarray(W.astype(ml_dtypes.bfloat16))

    # uwfw[f_lo, fc, j]: j<50 -> U_w[j, fc*128+f_lo];
    # j in [64,114) -> final_w[j-64, fc*128+f_lo]; rest zero
    UW = np.zeros((128, 2, 114), np.float32)
    UW[:, :, 0:L] = U_w.T.reshape(2, 128, L).transpose(1, 0, 2)
    UW[:, :, 64:64 + L] = final_w.T.reshape(2, 128, L).transpose(1, 0, 2)
    UW = np.ascontiguousarray(UW.astype(ml_dtypes.bfloat16))

    CB = np.ascontiguousarray(conv_b.reshape(2, 128).T.astype(np.float32))
    FB = np.ascontiguousarray(final_b.reshape(L, 1).astype(np.float32))
    return emb_bf, W, UW, CB, FB


def kernel(input_ids, emb_table, conv_w, conv_b, U_w, final_w, final_b):
    import os
    ids = np.asarray(input_ids)
    emb_table = np.asarray(emb_table, dtype=np.float32)
    conv_w = np.asarray(conv_w, dtype=np.float32)
    conv_b = np.asarray(conv_b, dtype=np.float32)
    U_w = np.asarray(U_w, dtype=np.float32)
    final_w = np.asarray(final_w, dtype=np.float32)
    final_b = np.asarray(final_b, dtype=np.float32)

    if "nc" not in _cache:
        _cache["nc"] = build_nc()
    nc = _cache["nc"]

    emb_bf, W, UW, CB, FB = _prep_shared(
        emb_table, conv_w, conv_b, U_w, final_w, final_b)

    ids16 = ids.astype(np.int16)  # vocab 30522 < 2**15
    in_maps = []
    for c in range(N_CORES):
        cid = ids16[c * BPC:(c + 1) * BPC]  # (BPC, S)
        # position i -> [i % 16, i // 16], batches along axis 1; the
        # 16-row block is replicated to all 8 gpsimd cores (128 rows)
        blk = np.concatenate(
            [cid[b].reshape(S // 16, 16).T for b in range(BPC)], axis=1)
        idx = np.tile(blk, (8, 1))
        in_maps.append({
            "emb": emb_bf, "idx": np.ascontiguousarray(idx),
            "wconv": W, "uwfw": UW, "cbias": CB, "fbias": FB,
        })

    trace = bool(int(os.environ.get("KERNEL_TRACE", "0")))
    res = run_bass_kernel_spmd(nc, in_maps, core_ids=list(range(N_CORES)),
                               trace=trace)
    _cache["last_result"] = res

    out = np.concatenate(
        [res.results[c]["out"].T for c in range(N_CORES)], axis=0)
    return np.ascontiguousarray(out.astype(np.float32))


# revision 12
# speedup vs baseline: 1.1217x; 1.0788x over previous
"""Trainium2 Bass kernel for nn_CAML_53240414601378.

Embedding lookup -> Conv1d(k=4, pad=2) -> tanh -> per-label attention
pooling -> logits. Data-parallel over batch across 8 NeuronCores
(4 batches per core); small params replicated.

Structure per batch (per core):
- dma_gather(transpose=True) pulls bf16 embedding rows straight into
  (e%128, e//128, s) layout -- no on-chip transpose. Batches 0 and 3
  gather in two halves so conv can start earlier / finish later against
  the serial SWDGE descriptor-generation chain.
- conv1d(k=4) = 4 shifted bf16 matmuls x 2 E-chunks x 2 F-chunks into
  PSUM; boundaries handled with shrunken-N matmuls (no padding).
- scores = U_w @ H and t = final_w @ H come from ONE matmul per
  (F-chunk, seq-tile) with a combined stationary operand (U_w -> psum
  rows 0..49, final_w -> rows 64..113).
- online softmax: per seq-tile partial (-max, Z, num) computed straight
  off the PSUM tile; tiny (50, 9) combine at the end. logits =
  sum_s alpha * t + final_b -- the (B,L,F) intermediate of the
  reference is never materialized.
"""

import numpy as np
import ml_dtypes

import concourse.bass as bass
import concourse.tile as tile
from concourse import bacc, mybir
from concourse.bass_utils import run_bass_kernel_spmd

B, S = 32, 4096
VOCAB, E, F, L = 30522, 256, 256, 50
SO = S + 1  # conv output length (4097)
N_CORES = 8
BPC = B // N_CORES  # batches per core
BF16 = mybir.dt.bfloat16
FP32 = mybir.dt.float32
NT, TN = 8, 512  # full seq tiles covering t in [0, 4096)
NJ = NT + 1      # score tiles (8x512 + 1)

_cache = {}


def _conv_mms(t0, n):
    """Conv matmul pieces for output cols [t0, t0+n): (k, lo, hi, off),
    full-width first so start=True covers the whole psum range."""
    shifts = []
    for k in range(4):
        lo = max(0, t0 + k - 2)
        hi = min(S, t0 + k - 2 + n)
        shifts.append((k, lo, hi, lo - (t0 + k - 2)))
    shifts.sort(key=lambda s: -(s[2] - s[1]))
    return shifts


def build_nc():
    nc = bacc.Bacc("TRN2", target_bir_lowering=False, debug=False,
                   num_devices=N_CORES)

    emb_ap = nc.dram_tensor("emb", (VOCAB, E), BF16, kind="ExternalInput").ap()
    idx_ap = nc.dram_tensor("idx", (128, BPC * S // 16), mybir.dt.int16,
                            kind="ExternalInput").ap()
    w_ap = nc.dram_tensor("wconv", (128, 16, 128), BF16,
                          kind="ExternalInput").ap()
    uw_ap = nc.dram_tensor("uwfw", (128, 2, 114), BF16,
                           kind="ExternalInput").ap()
    cb_ap = nc.dram_tensor("cbias", (128, 2), FP32, kind="ExternalInput").ap()
    fb_ap = nc.dram_tensor("fbias", (L, 1), FP32, kind="ExternalInput").ap()
    out_ap = nc.dram_tensor("out", (L, BPC), FP32, kind="ExternalOutput").ap()

    with tile.TileContext(nc) as tc:
        with (
            tc.tile_pool(name="const", bufs=1) as const,
            tc.tile_pool(name="xh", bufs=2) as xh,     # xa/xb halves
            tc.tile_pool(name="xp", bufs=2) as xp,     # full-batch x
            tc.tile_pool(name="hp", bufs=2) as hp,
            tc.tile_pool(name="ep", bufs=3) as ep,     # exp scratch tiles
            tc.tile_pool(name="pp", bufs=2) as pp,     # per-batch partials
            tc.tile_pool(name="small", bufs=8) as small,
            tc.tile_pool(name="psum", bufs=2, space="PSUM") as psum,
            tc.tile_pool(name="psum_st", bufs=4, space="PSUM") as psum_st,
        ):
            # ---- constants (loaded once) ----
            idx_sb = const.tile([128, BPC * S // 16], mybir.dt.int16)
            nc.sync.dma_start(idx_sb[:], idx_ap[:])
            w_sb = const.tile([128, 16, 128], BF16)
            nc.sync.dma_start(w_sb[:], w_ap[:])
            uw_sb = const.tile([128, 2, 114], BF16)
            nc.sync.dma_start(uw_sb[:], uw_ap[:])
            cb_sb = const.tile([128, 2], FP32)
            nc.sync.dma_start(cb_sb[:], cb_ap[:])
            fb_sb = const.tile([L, 1], FP32)
            nc.sync.dma_start(fb_sb[:], fb_ap[:])
            out_sb = const.tile([L, BPC], FP32)

            IPB = S // 16  # idx columns per batch
            HS = S // 2

            for b in range(BPC):
                # ---- embedding gather -> (e%128, e//128, s), bf16 ----
                if b in (0, BPC - 1):
                    xa = xh.tile([128, 2, HS], BF16, tag="xa")
                    xb = xh.tile([128, 2, HS], BF16, tag="xb")
                    for q, xt in ((0, xa), (1, xb)):
                        nc.gpsimd.dma_gather(
                            out_ap=xt[:], in_ap=emb_ap[:],
                            idxs_ap=idx_sb[:, b * IPB + q * (HS // 16):
                                           b * IPB + (q + 1) * (HS // 16)],
                            num_idxs=HS, num_idxs_reg=HS, elem_size=E,
                            transpose=True, single_packet=False)
                    segs = [(xa, 0, HS), (xb, HS, S)]
                else:
                    x_t = xp.tile([128, 2, S], BF16, tag="x")
                    nc.gpsimd.dma_gather(
                        out_ap=x_t[:], in_ap=emb_ap[:],
                        idxs_ap=idx_sb[:, b * IPB:(b + 1) * IPB],
                        num_idxs=S, num_idxs_reg=S, elem_size=E,
                        transpose=True, single_packet=False)
                    segs = [(x_t, 0, S)]

                def rhs_pieces(lo, hi):
                    out = []
                    for (xt, g0, g1) in segs:
                        a, bnd = max(lo, g0), min(hi, g1)
                        if a < bnd:
                            out.append((xt, a - g0, bnd - g0, a - lo))
                    return out

                # ---- conv1d(k=4) + bias + tanh -> H (f%128, f//128, t) ----
                H = hp.tile([128, 2, SO], BF16, tag="H")
                for fc in range(2):
                    for j in range(NT):
                        t0 = j * TN
                        ph = psum.tile([128, TN], FP32, tag=f"h{fc}")
                        mms = []
                        for (k, lo, hi, off) in _conv_mms(t0, TN):
                            for (xt, a, bnd, rel) in rhs_pieces(lo, hi):
                                for ec in range(2):
                                    mms.append((k, ec, xt, a, bnd, off + rel))
                        for i, (k, ec, xt, a, bnd, off) in enumerate(mms):
                            nc.tensor.matmul(
                                ph[:, off:off + (bnd - a)],
                                w_sb[:, k * 4 + ec * 2 + fc, :],
                                xt[:, ec, a:bnd],
                                start=(i == 0), stop=(i == len(mms) - 1),
                            )
                        nc.scalar.activation(
                            H[:, fc, t0:t0 + TN], ph[:],
                            mybir.ActivationFunctionType.Tanh,
                            bias=cb_sb[:, fc:fc + 1],
                        )
                    # last output column t = 4096 (x cols 4094..4095)
                    ph9 = psum.tile([128, 1], FP32, tag=f"h{fc}")
                    xt, g0, _ = segs[-1]
                    i = 0
                    for k in range(2):
                        for ec in range(2):
                            nc.tensor.matmul(
                                ph9[:, 0:1],
                                w_sb[:, k * 4 + ec * 2 + fc, :],
                                xt[:, ec, S - 2 + k - g0:S - 1 + k - g0],
                                start=(i == 0), stop=(i == 3),
                            )
                            i += 1
                    nc.scalar.activation(
                        H[:, fc, S:SO], ph9[:],
                        mybir.ActivationFunctionType.Tanh,
                        bias=cb_sb[:, fc:fc + 1],
                    )

                # ---- scores/t matmuls + online softmax partials ----
                nmx = pp.tile([L, NJ], FP32, tag="nmx")  # -max per tile
                zp = pp.tile([L, NJ], FP32, tag="zp")    # partial Z
                np_ = pp.tile([L, NJ], FP32, tag="np")   # partial num
                for j in range(NJ):
                    t0 = j * TN
                    n = TN if j < NT else 1
                    pst = psum_st.tile([114, n], FP32, tag="st")
                    for fc in range(2):
                        nc.tensor.matmul(
                            pst[:], uw_sb[:, fc, :], H[:, fc, t0:t0 + n],
                            start=(fc == 0), stop=(fc == 1),
                        )
                    nc.vector.reduce_max(nmx[:, j:j + 1], pst[0:L, :],
                                         axis=mybir.AxisListType.X,
                                         negate=True)
                    e_sb = ep.tile([L, TN], FP32, tag="e")
                    nc.scalar.activation(
                        e_sb[:, 0:n], pst[0:L, :],
                        mybir.ActivationFunctionType.Exp,
                        bias=nmx[:, j:j + 1], accum_out=zp[:, j:j + 1],
                    )
                    nc.vector.tensor_mul(e_sb[:, 0:n], e_sb[:, 0:n],
                                         pst[64:64 + L, :])
                    nc.vector.reduce_sum(np_[:, j:j + 1], e_sb[:, 0:n],
                                         axis=mybir.AxisListType.X)

                # ---- combine partials -> logits ----
                nm = small.tile([L, 1], FP32, tag="nm")  # -(global max)
                nc.vector.reduce_max(nm[:], nmx[:], axis=mybir.AxisListType.X,
                                     op=mybir.AluOpType.min)
                wj = small.tile([L, NJ], FP32, tag="wj")
                nc.scalar.activation(
                    wj[:], nmx[:], mybir.ActivationFunctionType.Exp,
                    bias=nm[:], scale=-1.0,
                )
                wz = small.tile([L, NJ], FP32, tag="wz")
                nc.vector.tensor_mul(wz[:], wj[:], zp[:])
                zsum = small.tile([L, 1], FP32, tag="zsum")
                nc.vector.reduce_sum(zsum[:], wz[:], axis=mybir.AxisListType.X)
                nc.vector.tensor_mul(wj[:], wj[:], np_[:])
                nsum = small.tile([L, 1], FP32, tag="nsum")
                nc.vector.reduce_sum(nsum[:], wj[:], axis=mybir.AxisListType.X)
                zr = small.tile([L, 1], FP32, tag="zr")
                nc.vector.reciprocal(zr[:], zsum[:])
                sm = small.tile([L, 1], FP32, tag="sm")
                nc.vector.tensor_mul(sm[:], nsum[:], zr[:])
                nc.vector.tensor_add(out_sb[:, b:b + 1], sm[:], fb_sb[:])

            nc.sync.dma_start(out_ap[:], out_sb[:])

    nc.compile()
    return nc


def _prep_shared(emb_table, conv_w, conv_b, U_w, final_w, final_b):
    emb_bf = np.ascontiguousarray(emb_table.astype(ml_dtypes.bfloat16))

    # wconv[e_lo, k*4 + ec*2 + fc, f_lo] = conv_w[fc*128+f, ec*128+e, k]
    W = np.empty((128, 16, 128), np.float32)
    for k in range(4):
        for ec in range(2):
            for fc in range(2):
                W[:, k * 4 + ec * 2 + fc, :] = conv_w[
                    fc * 128:(fc + 1) * 128, ec * 128:(ec + 1) * 128, k].T
    W = np.ascontiguousarray(W.astype(ml_dtypes.bfloat16))

    # uwfw[f_lo, fc, j]: j<50 -> U_w[j, fc*128+f_lo];
    # j in [64,114) -> final_w[j-64, fc*128+f_lo]; rest zero
    UW = np.zeros((128, 2, 114), np.float32)
    UW[:, :, 0:L] = U_w.T.reshape(2, 128, L).transpose(1, 0, 2)
    UW[:, :, 64:64 + L] = final_w.T.reshape(2, 128, L).transpose(1, 0, 2)
    UW = np.ascontiguousarray(UW.astype(ml_dtypes.bfloat16))

    CB = np.ascontiguousarray(conv_b.reshape(2, 128).T.astype(np.float32))
    FB = np.ascontiguousarray(final_b.reshape(L, 1).astype(np.float32))
    return emb_bf, W, UW, CB, FB


def kernel(input_ids, emb_table, conv_w, conv_b, U_w, final_w, final_b):
    import os
    ids = np.asarray(input_ids)
    emb_table = np.asarray(emb_table, dtype=np.float32)
    conv_w = np.asarray(conv_w, dtype=np.float32)
    conv_b = np.asarray(conv_b, dtype=np.float32)
    U_w = np.asarray(U_w, dtype=np.float32)
    final_w = np.asarray(final_w, dtype=np.float32)
    final_b = np.asarray(final_b, dtype=np.float32)

    if "nc" not in _cache:
        _cache["nc"] = build_nc()
    nc = _cache["nc"]

    emb_bf, W, UW, CB, FB = _prep_shared(
        emb_table, conv_w, conv_b, U_w, final_w, final_b)

    ids16 = ids.astype(np.int16)  # vocab 30522 < 2**15
    in_maps = []
    for c in range(N_CORES):
        cid = ids16[c * BPC:(c + 1) * BPC]  # (BPC, S)
        # position i -> [i % 16, i // 16], batches along axis 1; the
        # 16-row block is replicated to all 8 gpsimd cores (128 rows)
        blk = np.concatenate(
            [cid[b].reshape(S // 16, 16).T for b in range(BPC)], axis=1)
        idx = np.tile(blk, (8, 1))
        in_maps.append({
            "emb": emb_bf, "idx": np.ascontiguousarray(idx),
            "wconv": W, "uwfw": UW, "cbias": CB, "fbias": FB,
        })

    trace = bool(int(os.environ.get("KERNEL_TRACE", "0")))
    res = run_bass_kernel_spmd(nc, in_maps, core_ids=list(range(N_CORES)),
                               trace=trace)
    _cache["last_result"] = res

    out = np.concatenate(
        [res.results[c]["out"].T for c in range(N_CORES)], axis=0)
    return np.ascontiguousarray(out.astype(np.float32))


# revision 13
# speedup vs baseline: 1.2309x; 1.0974x over previous
"""Trainium2 Bass kernel for nn_CAML_53240414601378.

Embedding lookup -> Conv1d(k=4, pad=2) -> tanh -> per-label attention
pooling -> logits. Data-parallel over batch across 8 NeuronCores
(4 batches per core); small params replicated.

Structure per batch (per core):
- dma_gather(transpose=True) pulls bf16 embedding rows straight into
  (e%128, e//128, s) layout -- no on-chip transpose. Batches 0 and 3
  gather in two halves so conv can start earlier / finish later against
  the serial SWDGE descriptor-generation chain.
- conv1d(k=4) = 4 shifted bf16 matmuls x 2 E-chunks x 2 F-chunks into
  PSUM; boundaries handled with shrunken-N matmuls (no padding).
- scores = U_w @ H and t = final_w @ H come from ONE matmul per
  (F-chunk, seq-tile) with a combined stationary operand (U_w -> psum
  rows 0..49, final_w -> rows 64..113).
- online softmax: per seq-tile partial (-max, Z, num) computed straight
  off the PSUM tile; tiny (50, 9) combine at the end. logits =
  sum_s alpha * t + final_b -- the (B,L,F) intermediate of the
  reference is never materialized.
"""

import numpy as np
import ml_dtypes

import concourse.bass as bass
import concourse.tile as tile
from concourse import bacc, mybir
from concourse.bass_utils import run_bass_kernel_spmd

B, S = 32, 4096
VOCAB, E, F, L = 30522, 256, 256, 50
SO = S + 1  # conv output length (4097)
N_CORES = 8
BPC = B // N_CORES  # batches per core
BF16 = mybir.dt.bfloat16
FP32 = mybir.dt.float32
NT, TN = 8, 512  # full seq tiles covering t in [0, 4096)
NJ = NT + 1      # score tiles (8x512 + 1)

_cache = {}


def _conv_mms(t0, n):
    """Conv matmul pieces for output cols [t0, t0+n): (k, lo, hi, off),
    full-width first so start=True covers the whole psum range."""
    shifts = []
    for k in range(4):
        lo = max(0, t0 + k - 2)
        hi = min(S, t0 + k - 2 + n)
        shifts.append((k, lo, hi, lo - (t0 + k - 2)))
    shifts.sort(key=lambda s: -(s[2] - s[1]))
    return shifts


def build_nc():
    nc = bacc.Bacc("TRN2", target_bir_lowering=False, debug=False,
                   num_devices=N_CORES)

    emb_ap = nc.dram_tensor("emb", (VOCAB, E), BF16, kind="ExternalInput").ap()
    idx_ap = nc.dram_tensor("idx", (128, BPC * S // 16), mybir.dt.int16,
                            kind="ExternalInput").ap()
    w_ap = nc.dram_tensor("wconv", (128, 16, 128), BF16,
                          kind="ExternalInput").ap()
    uw_ap = nc.dram_tensor("uwfw", (128, 2, 114), BF16,
                           kind="ExternalInput").ap()
    cb_ap = nc.dram_tensor("cbias", (128, 2), FP32, kind="ExternalInput").ap()
    fb_ap = nc.dram_tensor("fbias", (L, 1), FP32, kind="ExternalInput").ap()
    out_ap = nc.dram_tensor("out", (L, BPC), FP32, kind="ExternalOutput").ap()

    with tile.TileContext(nc) as tc:
        with (
            tc.tile_pool(name="const", bufs=1) as const,
            tc.tile_pool(name="xh", bufs=3) as xh,     # gather half-tiles
            tc.tile_pool(name="xp", bufs=2) as xp,     # full-batch x
            tc.tile_pool(name="hp", bufs=2) as hp,
            tc.tile_pool(name="ep", bufs=3) as ep,     # exp scratch tiles
            tc.tile_pool(name="pp", bufs=2) as pp,     # per-batch partials
            tc.tile_pool(name="small", bufs=8) as small,
            tc.tile_pool(name="psum", bufs=2, space="PSUM") as psum,
            tc.tile_pool(name="psum_st", bufs=4, space="PSUM") as psum_st,
        ):
            # ---- constants (loaded once) ----
            idx_sb = const.tile([128, BPC * S // 16], mybir.dt.int16)
            nc.sync.dma_start(idx_sb[:], idx_ap[:])
            w_sb = const.tile([128, 16, 128], BF16)
            nc.sync.dma_start(w_sb[:], w_ap[:])
            uw_sb = const.tile([128, 2, 114], BF16)
            nc.sync.dma_start(uw_sb[:], uw_ap[:])
            cb_sb = const.tile([128, 2], FP32)
            nc.sync.dma_start(cb_sb[:], cb_ap[:])
            fb_sb = const.tile([L, 1], FP32)
            nc.sync.dma_start(fb_sb[:], fb_ap[:])
            out_sb = const.tile([L, BPC], FP32)

            IPB = S // 16  # idx columns per batch
            HS = S // 2

            for b in range(BPC):
                # ---- embedding gather -> (e%128, e//128, s), bf16 ----
                # two halves per batch: conv starts on half A while the
                # serial SWDGE descriptor generation works on half B
                xa = xh.tile([128, 2, HS], BF16, tag="xa")
                xb = xh.tile([128, 2, HS], BF16, tag="xb")
                for q, xt in ((0, xa), (1, xb)):
                    nc.gpsimd.dma_gather(
                        out_ap=xt[:], in_ap=emb_ap[:],
                        idxs_ap=idx_sb[:, b * IPB + q * (HS // 16):
                                       b * IPB + (q + 1) * (HS // 16)],
                        num_idxs=HS, num_idxs_reg=HS, elem_size=E,
                        transpose=True, single_packet=False)
                segs = [(xa, 0, HS), (xb, HS, S)]

                def rhs_pieces(lo, hi):
                    out = []
                    for (xt, g0, g1) in segs:
                        a, bnd = max(lo, g0), min(hi, g1)
                        if a < bnd:
                            out.append((xt, a - g0, bnd - g0, a - lo))
                    return out

                H = hp.tile([128, 2, SO], BF16, tag="H")
                nmx = pp.tile([L, NJ], FP32, tag="nmx")  # -max per tile
                zp = pp.tile([L, NJ], FP32, tag="zp")    # partial Z
                np_ = pp.tile([L, NJ], FP32, tag="np")   # partial num

                def score_tile(j, n):
                    """Combined scores/t matmul for H cols [j*TN, +n) and
                    the online-softmax partials for that tile."""
                    t0 = j * TN
                    pst = psum_st.tile([114, TN], FP32, tag="st")
                    for fc in range(2):
                        nc.tensor.matmul(
                            pst[:, 0:n], uw_sb[:, fc, :], H[:, fc, t0:t0 + n],
                            start=(fc == 0), stop=(fc == 1),
                        )
                    nc.vector.reduce_max(nmx[:, j:j + 1], pst[0:L, 0:n],
                                         axis=mybir.AxisListType.X,
                                         negate=True)
                    e_sb = ep.tile([L, TN], FP32, tag="e")
                    nc.scalar.activation(
                        e_sb[:, 0:n], pst[0:L, 0:n],
                        mybir.ActivationFunctionType.Exp,
                        bias=nmx[:, j:j + 1], accum_out=zp[:, j:j + 1],
                    )
                    nc.vector.tensor_mul(e_sb[:, 0:n], e_sb[:, 0:n],
                                         pst[64:64 + L, 0:n])
                    nc.vector.reduce_sum(np_[:, j:j + 1], e_sb[:, 0:n],
                                         axis=mybir.AxisListType.X)

                # last output column t = 4096 (x cols 4094..4095) first,
                # so the j=8 score tile is off the critical tail
                for fc in range(2):
                    ph9 = psum.tile([128, 1], FP32, tag=f"h{fc}")
                    i = 0
                    for k in range(2):
                        for ec in range(2):
                            nc.tensor.matmul(
                                ph9[:, 0:1],
                                w_sb[:, k * 4 + ec * 2 + fc, :],
                                xb[:, ec, HS - 2 + k:HS - 1 + k],
                                start=(i == 0), stop=(i == 3),
                            )
                            i += 1
                    nc.scalar.activation(
                        H[:, fc, S:SO], ph9[:],
                        mybir.ActivationFunctionType.Tanh,
                        bias=cb_sb[:, fc:fc + 1],
                    )
                score_tile(NT, 1)

                # ---- conv1d(k=4) + bias + tanh + scores, per seq tile ----
                for j in range(NT):
                    t0 = j * TN
                    for fc in range(2):
                        ph = psum.tile([128, TN], FP32, tag=f"h{fc}")
                        mms = []
                        for (k, lo, hi, off) in _conv_mms(t0, TN):
                            for (xt, a, bnd, rel) in rhs_pieces(lo, hi):
                                for ec in range(2):
                                    mms.append((k, ec, xt, a, bnd, off + rel))
                        for i, (k, ec, xt, a, bnd, off) in enumerate(mms):
                            nc.tensor.matmul(
                                ph[:, off:off + (bnd - a)],
                                w_sb[:, k * 4 + ec * 2 + fc, :],
                                xt[:, ec, a:bnd],
                                start=(i == 0), stop=(i == len(mms) - 1),
                            )
                        nc.scalar.activation(
                            H[:, fc, t0:t0 + TN], ph[:],
                            mybir.ActivationFunctionType.Tanh,
                            bias=cb_sb[:, fc:fc + 1],
                        )
                    score_tile(j, TN)

                # ---- combine partials -> logits ----
                nm = small.tile([L, 1], FP32, tag="nm")  # -(global max)
                nc.vector.reduce_max(nm[:], nmx[:], axis=mybir.AxisListType.X,
                                     op=mybir.AluOpType.min)
                wj = small.tile([L, NJ], FP32, tag="wj")
                nc.scalar.activation(
                    wj[:], nmx[:], mybir.ActivationFunctionType.Exp,
                    bias=nm[:], scale=-1.0,
                )
                wz = small.tile([L, NJ], FP32, tag="wz")
                nc.vector.tensor_mul(wz[:], wj[:], zp[:])
                zsum = small.tile([L, 1], FP32, tag="zsum")
                nc.vector.reduce_sum(zsum[:], wz[:], axis=mybir.AxisListType.X)
                nc.vector.tensor_mul(wj[:], wj[:], np_[:])
                nsum = small.tile([L, 1], FP32, tag="nsum")
                nc.vector.reduce_sum(nsum[:], wj[:], axis=mybir.AxisListType.X)
                zr = small.tile([L, 1], FP32, tag="zr")
                nc.vector.reciprocal(zr[:], zsum[:])
                sm = small.tile([L, 1], FP32, tag="sm")
                nc.vector.tensor_mul(sm[:], nsum[:], zr[:])
                nc.vector.tensor_add(out_sb[:, b:b + 1], sm[:], fb_sb[:])

            nc.sync.dma_start(out_ap[:], out_sb[:])

    nc.compile()
    return nc


def _prep_shared(emb_table, conv_w, conv_b, U_w, final_w, final_b):
    emb_bf = np.ascontiguousarray(emb_table.astype(ml_dtypes.bfloat16))

    # wconv[e_lo, k*4 + ec*2 + fc, f_lo] = conv_w[fc*128+f, ec*128+e, k]
    W = np.empty((128, 16, 128), np.float32)
    for k in range(4):
        for ec in range(2):
            for fc in range(2):
                W[:, k * 4 + ec * 2 + fc, :] = conv_w[
                    fc * 128:(fc + 1) * 128, ec * 128:(ec + 1) * 128, k].T
    W = np.ascontiguousarray(W.astype(ml_dtypes.bfloat16))

    # uwfw[f_lo, fc, j]: j<50 -> U_w[j, fc*128+f_lo];
    # j in [64,114) -> final_w[j-64, fc*128+f_lo]; rest zero
    UW = np.zeros((128, 2, 114), np.float32)
    UW[:, :, 0:L] = U_w.T.reshape(2, 128, L).transpose(1, 0, 2)
    UW[:, :, 64:64 + L] = final_w.T.reshape(2, 128, L).transpose(1, 0, 2)
    UW = np.ascontiguousarray(UW.astype(ml_dtypes.bfloat16))

    CB = np.ascontiguousarray(conv_b.reshape(2, 128).T.astype(np.float32))
    FB = np.ascontiguousarray(final_b.reshape(L, 1).astype(np.float32))
    return emb_bf, W, UW, CB, FB


def kernel(input_ids, emb_table, conv_w, conv_b, U_w, final_w, final_b):
    import os
    ids = np.asarray(input_ids)
    emb_table = np.asarray(emb_table, dtype=np.float32)
    conv_w = np.asarray(conv_w, dtype=np.float32)
    conv_b = np.asarray(conv_b, dtype=np.float32)
    U_w = np.asarray(U_w, dtype=np.float32)
    final_w = np.asarray(final_w, dtype=np.float32)
    final_b = np.asarray(final_b, dtype=np.float32)

    if "nc" not in _cache:
        _cache["nc"] = build_nc()
    nc = _cache["nc"]

    emb_bf, W, UW, CB, FB = _prep_shared(
        emb_table, conv_w, conv_b, U_w, final_w, final_b)

    ids16 = ids.astype(np.int16)  # vocab 30522 < 2**15
    in_maps = []
    for c in range(N_CORES):
        cid = ids16[c * BPC:(c + 1) * BPC]  # (BPC, S)
        # position i -> [i % 16, i // 16], batches along axis 1; the
        # 16-row block is replicated to all 8 gpsimd cores (128 rows)
        blk = np.concatenate(
            [cid[b].reshape(S // 16, 16).T for b in range(BPC)], axis=1)
        idx = np.tile(blk, (8, 1))
        in_maps.append({
            "emb": emb_bf, "idx": np.ascontiguousarray(idx),
            "wconv": W, "uwfw": UW, "cbias": CB, "fbias": FB,
        })

    trace = bool(int(os.environ.get("KERNEL_TRACE", "0")))
    res = run_bass_kernel_spmd(nc, in_maps, core_ids=list(range(N_CORES)),
                               trace=trace)
    _cache["last_result"] = res

    out = np.concatenate(
        [res.results[c]["out"].T for c in range(N_CORES)], axis=0)
    return np.ascontiguousarray(out.astype(np.float32))


# revision 14
# speedup vs baseline: 1.3233x; 1.0751x over previous
"""Trainium2 Bass kernel for nn_CAML_53240414601378.

Embedding lookup -> Conv1d(k=4, pad=2) -> tanh -> per-label attention
pooling -> logits. Data-parallel over batch across 8 NeuronCores
(4 batches per core); small params replicated.

Structure per batch (per core):
- dma_gather(transpose=True) pulls bf16 embedding rows straight into
  (e%128, e//128, s) layout -- no on-chip transpose. Batches 0 and 3
  gather in two halves so conv can start earlier / finish later against
  the serial SWDGE descriptor-generation chain.
- conv1d(k=4) = 4 shifted bf16 matmuls x 2 E-chunks x 2 F-chunks into
  PSUM; boundaries handled with shrunken-N matmuls (no padding).
- scores = U_w @ H and t = final_w @ H come from ONE matmul per
  (F-chunk, seq-tile) with a combined stationary operand (U_w -> psum
  rows 0..49, final_w -> rows 64..113).
- online softmax: per seq-tile partial (-max, Z, num) computed straight
  off the PSUM tile; tiny (50, 9) combine at the end. logits =
  sum_s alpha * t + final_b -- the (B,L,F) intermediate of the
  reference is never materialized.
"""

import numpy as np
import ml_dtypes

import concourse.bass as bass
import concourse.tile as tile
from concourse import bacc, mybir
from concourse.bass_utils import run_bass_kernel_spmd

B, S = 32, 4096
VOCAB, E, F, L = 30522, 256, 256, 50
SO = S + 1  # conv output length (4097)
N_CORES = 8
BPC = B // N_CORES  # batches per core
BF16 = mybir.dt.bfloat16
FP32 = mybir.dt.float32
NT, TN = 8, 512  # full seq tiles covering t in [0, 4096)
NJ = NT + 1      # score tiles (8x512 + 1)

_cache = {}


def _conv_mms(t0, n):
    """Conv matmul pieces for output cols [t0, t0+n): (k, lo, hi, off),
    full-width first so start=True covers the whole psum range."""
    shifts = []
    for k in range(4):
        lo = max(0, t0 + k - 2)
        hi = min(S, t0 + k - 2 + n)
        shifts.append((k, lo, hi, lo - (t0 + k - 2)))
    shifts.sort(key=lambda s: -(s[2] - s[1]))
    return shifts


def build_nc():
    nc = bacc.Bacc("TRN2", target_bir_lowering=False, debug=False,
                   num_devices=N_CORES)

    emb_ap = nc.dram_tensor("emb", (VOCAB, E), BF16, kind="ExternalInput").ap()
    idx_ap = nc.dram_tensor("idx", (128, BPC * S // 16), mybir.dt.int16,
                            kind="ExternalInput").ap()
    w_ap = nc.dram_tensor("wconv", (128, 16, 128), BF16,
                          kind="ExternalInput").ap()
    uw_ap = nc.dram_tensor("uwfw", (128, 2, 114), BF16,
                           kind="ExternalInput").ap()
    cb_ap = nc.dram_tensor("cbias", (128, 2), FP32, kind="ExternalInput").ap()
    fb_ap = nc.dram_tensor("fbias", (L, 1), FP32, kind="ExternalInput").ap()
    out_ap = nc.dram_tensor("out", (L, BPC), FP32, kind="ExternalOutput").ap()

    with tile.TileContext(nc) as tc:
        with (
            tc.tile_pool(name="const", bufs=1) as const,
            tc.tile_pool(name="xh", bufs=3) as xh,     # gather half-tiles
            tc.tile_pool(name="xp", bufs=2) as xp,     # full-batch x
            tc.tile_pool(name="hp", bufs=2) as hp,
            tc.tile_pool(name="ep", bufs=3) as ep,     # exp scratch tiles
            tc.tile_pool(name="pp", bufs=2) as pp,     # per-batch partials
            tc.tile_pool(name="small", bufs=8) as small,
            tc.tile_pool(name="psum", bufs=2, space="PSUM") as psum,
            tc.tile_pool(name="psum_st", bufs=4, space="PSUM") as psum_st,
        ):
            # ---- constants (loaded once) ----
            idx_sb = const.tile([128, BPC * S // 16], mybir.dt.int16)
            nc.gpsimd.dma_start(idx_sb[:], idx_ap[:])
            w_sb = const.tile([128, 16, 128], BF16)
            nc.sync.dma_start(w_sb[:], w_ap[:])
            uw_sb = const.tile([128, 2, 114], BF16)
            nc.sync.dma_start(uw_sb[:], uw_ap[:])
            cb_sb = const.tile([128, 2], FP32)
            nc.sync.dma_start(cb_sb[:], cb_ap[:])
            fb_sb = const.tile([L, 1], FP32)
            nc.sync.dma_start(fb_sb[:], fb_ap[:])
            out_sb = const.tile([L, BPC], FP32)

            IPB = S // 16  # idx columns per batch
            HS = S // 2

            for b in range(BPC):
                # ---- embedding gather -> (e%128, e//128, s), bf16 ----
                # two halves per batch: conv starts on half A while the
                # serial SWDGE descriptor generation works on half B
                nq = 4 if b in (0, BPC - 1) else 2
                CS = S // nq
                segs = []
                for q in range(nq):
                    xt = xh.tile([128, 2, CS], BF16, tag=f"x{nq}_{q}")
                    nc.gpsimd.dma_gather(
                        out_ap=xt[:], in_ap=emb_ap[:],
                        idxs_ap=idx_sb[:, b * IPB + q * (CS // 16):
                                       b * IPB + (q + 1) * (CS // 16)],
                        num_idxs=CS, num_idxs_reg=CS, elem_size=E,
                        transpose=True, single_packet=False)
                    segs.append((xt, q * CS, (q + 1) * CS))

                def rhs_pieces(lo, hi):
                    out = []
                    for (xt, g0, g1) in segs:
                        a, bnd = max(lo, g0), min(hi, g1)
                        if a < bnd:
                            out.append((xt, a - g0, bnd - g0, a - lo))
                    return out

                H = hp.tile([128, 2, SO], BF16, tag="H")
                nmx = pp.tile([L, NJ], FP32, tag="nmx")  # -max per tile
                zp = pp.tile([L, NJ], FP32, tag="zp")    # partial Z
                np_ = pp.tile([L, NJ], FP32, tag="np")   # partial num

                def score_tile(j, n):
                    """Combined scores/t matmul for H cols [j*TN, +n) and
                    the online-softmax partials for that tile."""
                    t0 = j * TN
                    pst = psum_st.tile([114, TN], FP32, tag="st")
                    for fc in range(2):
                        nc.tensor.matmul(
                            pst[:, 0:n], uw_sb[:, fc, :], H[:, fc, t0:t0 + n],
                            start=(fc == 0), stop=(fc == 1),
                        )
                    nc.vector.reduce_max(nmx[:, j:j + 1], pst[0:L, 0:n],
                                         axis=mybir.AxisListType.X,
                                         negate=True)
                    e_sb = ep.tile([L, TN], FP32, tag="e")
                    nc.scalar.activation(
                        e_sb[:, 0:n], pst[0:L, 0:n],
                        mybir.ActivationFunctionType.Exp,
                        bias=nmx[:, j:j + 1], accum_out=zp[:, j:j + 1],
                    )
                    nc.vector.tensor_mul(e_sb[:, 0:n], e_sb[:, 0:n],
                                         pst[64:64 + L, 0:n])
                    nc.vector.reduce_sum(np_[:, j:j + 1], e_sb[:, 0:n],
                                         axis=mybir.AxisListType.X)

                # ---- conv1d(k=4) + bias + tanh + scores, per seq tile ----
                for j in range(NT):
                    t0 = j * TN
                    for fc in range(2):
                        ph = psum.tile([128, TN], FP32, tag=f"h{fc}")
                        mms = []
                        for (k, lo, hi, off) in _conv_mms(t0, TN):
                            for (xt, a, bnd, rel) in rhs_pieces(lo, hi):
                                for ec in range(2):
                                    mms.append((k, ec, xt, a, bnd, off + rel))
                        for i, (k, ec, xt, a, bnd, off) in enumerate(mms):
                            nc.tensor.matmul(
                                ph[:, off:off + (bnd - a)],
                                w_sb[:, k * 4 + ec * 2 + fc, :],
                                xt[:, ec, a:bnd],
                                start=(i == 0), stop=(i == len(mms) - 1),
                            )
                        nc.scalar.activation(
                            H[:, fc, t0:t0 + TN], ph[:],
                            mybir.ActivationFunctionType.Tanh,
                            bias=cb_sb[:, fc:fc + 1],
                        )
                    score_tile(j, TN)

                # last output column t = 4096 (x cols 4094..4095)
                for fc in range(2):
                    ph9 = psum.tile([128, 1], FP32, tag=f"h{fc}")
                    i = 0
                    for k in range(2):
                        for ec in range(2):
                            nc.tensor.matmul(
                                ph9[:, 0:1],
                                w_sb[:, k * 4 + ec * 2 + fc, :],
                                segs[-1][0][:, ec, CS - 2 + k:CS - 1 + k],
                                start=(i == 0), stop=(i == 3),
                            )
                            i += 1
                    nc.scalar.activation(
                        H[:, fc, S:SO], ph9[:],
                        mybir.ActivationFunctionType.Tanh,
                        bias=cb_sb[:, fc:fc + 1],
                    )
                score_tile(NT, 1)


                # ---- combine partials -> logits ----
                nm = small.tile([L, 1], FP32, tag="nm")  # -(global max)
                nc.vector.reduce_max(nm[:], nmx[:], axis=mybir.AxisListType.X,
                                     op=mybir.AluOpType.min)
                wj = small.tile([L, NJ], FP32, tag="wj")
                nc.scalar.activation(
                    wj[:], nmx[:], mybir.ActivationFunctionType.Exp,
                    bias=nm[:], scale=-1.0,
                )
                wz = small.tile([L, NJ], FP32, tag="wz")
                nc.vector.tensor_mul(wz[:], wj[:], zp[:])
                zsum = small.tile([L, 1], FP32, tag="zsum")
                nc.vector.reduce_sum(zsum[:], wz[:], axis=mybir.AxisListType.X)
                nc.vector.tensor_mul(wj[:], wj[:], np_[:])
                nsum = small.tile([L, 1], FP32, tag="nsum")
                nc.vector.reduce_sum(nsum[:], wj[:], axis=mybir.AxisListType.X)
                zr = small.tile([L, 1], FP32, tag="zr")
                nc.vector.reciprocal(zr[:], zsum[:])
                sm = small.tile([L, 1], FP32, tag="sm")
                nc.vector.tensor_mul(sm[:], nsum[:], zr[:])
                nc.vector.tensor_add(out_sb[:, b:b + 1], sm[:], fb_sb[:])

            nc.sync.dma_start(out_ap[:], out_sb[:])

    nc.compile()
    return nc


def _prep_shared(emb_table, conv_w, conv_b, U_w, final_w, final_b):
    emb_bf = np.ascontiguousarray(emb_table.astype(ml_dtypes.bfloat16))

    # wconv[e_lo, k*4 + ec*2 + fc, f_lo] = conv_w[fc*128+f, ec*128+e, k]
    W = np.empty((128, 16, 128), np.float32)
    for k in range(4):
        for ec in range(2):
            for fc in range(2):
                W[:, k * 4 + ec * 2 + fc, :] = conv_w[
                    fc * 128:(fc + 1) * 128, ec * 128:(ec + 1) * 128, k].T
    W = np.ascontiguousarray(W.astype(ml_dtypes.bfloat16))

    # uwfw[f_lo, fc, j]: j<50 -> U_w[j, fc*128+f_lo];
    # j in [64,114) -> final_w[j-64, fc*128+f_lo]; rest zero
    UW = np.zeros((128, 2, 114), np.float32)
    UW[:, :, 0:L] = U_w.T.reshape(2, 128, L).transpose(1, 0, 2)
    UW[:, :, 64:64 + L] = final_w.T.reshape(2, 128, L).transpose(1, 0, 2)
    UW = np.ascontiguousarray(UW.astype(ml_dtypes.bfloat16))

    CB = np.ascontiguousarray(conv_b.reshape(2, 128).T.astype(np.float32))
    FB = np.ascontiguousarray(final_b.reshape(L, 1).astype(np.float32))
    return emb_bf, W, UW, CB, FB


def kernel(input_ids, emb_table, conv_w, conv_b, U_w, final_w, final_b):
    import os
    ids = np.asarray(input_ids)
    emb_table = np.asarray(emb_table, dtype=np.float32)
    conv_w = np.asarray(conv_w, dtype=np.float32)
    conv_b = np.asarray(conv_b, dtype=np.float32)
    U_w = np.asarray(U_w, dtype=np.float32)
    final_w = np.asarray(final_w, dtype=np.float32)
    final_b = np.asarray(final_b, dtype=np.float32)

    if "nc" not in _cache:
        _cache["nc"] = build_nc()
    nc = _cache["nc"]

    emb_bf, W, UW, CB, FB = _prep_shared(
        emb_table, conv_w, conv_b, U_w, final_w, final_b)

    ids16 = ids.astype(np.int16)  # vocab 30522 < 2**15
    in_maps = []
    for c in range(N_CORES):
        cid = ids16[c * BPC:(c + 1) * BPC]  # (BPC, S)
        # position i -> [i % 16, i // 16], batches along axis 1; the
        # 16-row block is replicated to all 8 gpsimd cores (128 rows)
        blk = np.concatenate(
            [cid[b].reshape(S // 16, 16).T for b in range(BPC)], axis=1)
        idx = np.tile(blk, (8, 1))
        in_maps.append({
            "emb": emb_bf, "idx": np.ascontiguousarray(idx),
            "wconv": W, "uwfw": UW, "cbias": CB, "fbias": FB,
        })

    trace = bool(int(os.environ.get("KERNEL_TRACE", "0")))
    res = run_bass_kernel_spmd(nc, in_maps, core_ids=list(range(N_CORES)),
                               trace=trace)
    _cache["last_result"] = res

    out = np.concatenate(
        [res.results[c]["out"].T for c in range(N_CORES)], axis=0)
    return np.ascontiguousarray(out.astype(np.float32))
